# revision 50
# baseline (speedup 1.0000x reference)
"""AttentionSequencePoolingLayer Trainium2 kernel (8-core data parallel), v2.

B=2048, S=200, D=64, H1=64, H2=16. Batch sharded 256/core.

Strategy:
- Rows are globally sorted by seq_length and dealt round-robin to the 8 cores,
  so every core sees the same length profile. Within a core, rows are grouped
  16 at a time; group g only processes T_g = roundup(max seq_length, 16)
  tokens (ΣT ≈ 0.57 × S). One program (compiled per schedule) serves all cores.
- k is re-laid token-major per 64-row block on the host (8KB DMA descriptors,
  zero-padded to 256 tokens); cast-load bf16 (tokens on partitions), xbar-transpose
  to kT [(bhat,d), tok] pair tiles.
- z1 = x1 = att@W1 lands in PSUM via 3 accumulating matmuls per pair:
  qW ⊗ ones (K=1 rank-1), Wk^T kT, (q∘Wqk)^T kT. The qW fold makes the
  dice-1 sigmoid bias a shared constant, so ACT/DVE ops batch across pairs.
- dice1, two routes (mixed 3:8 r2b:r1 to balance ACT vs DVE):
  r1: ACT p1=σ(s·x1+b); DVE p1~=α+(1-α)p1; DVE h1=x1(PSUM)∘p1~; z2=W2^T h1.
  r2b: ACT p1; ACT copy x1→SBUF; DVE u1=x1∘p1; z2 = (αW2)^T x1 + ((1-α)W2)^T u1.
- dice2 batched over all 8 pairs of a group [128=(pair,bhat,h2), 2T].
- scores via tiny N=2 matmuls into token-major PSUM; σ+mask; pooling as
  out[d,1] = k_nat^T w with N=1 matmuls (k stationary); d-major DRAM output,
  transposed on host at gather.
"""
import numpy as np
import ml_dtypes

import concourse.bacc as bacc
import concourse.tile as tile
import concourse.mybir as mybir
import concourse.bass as bass
from concourse.bass_utils import run_bass_kernel_spmd

B, S, D = 2048, 200, 64
H1, H2 = 64, 16
EPS = 1e-9
NCORES = 8
BLOC = B // NCORES          # 256 rows per core
NGROUPS = BLOC // 16        # 16

F32 = mybir.dt.float32
BF16 = mybir.dt.bfloat16
AF = mybir.ActivationFunctionType
ALU = mybir.AluOpType
bf = ml_dtypes.bfloat16

# fraction of z1-units on route-2b (ACT-heavy) vs route-1 (DVE-heavy): num/den
R2B_NUM, R2B_DEN = 3, 8
LP_BUFS, KT_BUFS, WP_BUFS, H2_BUFS = 2, 6, 6, 3
PS1_BUFS, PS2_BUFS, PS3_BUFS = 3, 2, 3
WQ_ACT_MOD = 9  # j%4 value routed to ACT; 9=never
UP8, UP4 = 56, 112
WQ_POOL_MOD = 9
PACK_CONSTS = False
HOIST_WQ = False
HOIST_N = 16
POCOPY_ACT = False
H2B_MOD, H2B_PHASE = 99, 98
WQH_START = 99  # groups >= this use host-precomputed q*Wqk (disabled: adds HWDGE sem pressure)
COMPUTE_ORDER = ()
LOAD_BLOCKS = ((0, 4), (4, 4), (8, 4), (12, 4))  # (start_group, ngroups)

_CACHE = {}
TRACE = False
LAST_RESULT = None


def _ceil16(x):
    return (int(x) + 15) // 16 * 16


def _build(sched):
    nc = bacc.Bacc("TRN2", target_bir_lowering=False, debug=False, num_devices=NCORES,
                   dynamic_dma_scratch_size=16384)
    nb = BLOC
    npair = nb // 2  # 128

    ST = 256  # padded tokens per load block (two 128-token chunks)
    keyt = nc.dram_tensor("keyt", [4 * ST * 64, D], F32, kind="ExternalInput").ap()
    qp = nc.dram_tensor("qp", [128, npair], F32, kind="ExternalInput").ap()
    qw1 = nc.dram_tensor("qw1", [1, 128 * npair], BF16, kind="ExternalInput").ap()
    maskd = nc.dram_tensor("maskd", [128, 32 * NGROUPS], BF16, kind="ExternalInput").ap()
    wk2 = nc.dram_tensor("wk2", [128, 128], BF16, kind="ExternalInput").ap()
    wqk2 = nc.dram_tensor("wqk2", [128, 128], BF16, kind="ExternalInput").ap()
    w2b = nc.dram_tensor("w2b", [128, 32], BF16, kind="ExternalInput").ap()
    w2a = nc.dram_tensor("w2a", [128, 32], BF16, kind="ExternalInput").ap()
    w2na = nc.dram_tensor("w2na", [128, 32], BF16, kind="ExternalInput").ap()
    w34 = nc.dram_tensor("w34", [128, 2], BF16, kind="ExternalInput").ap()
    colsb = nc.dram_tensor("colsb", [128, 8], F32, kind="ExternalInput").ap()
    NBF = 32 * NGROUPS + 128 + 128 + 32 + 32 + 32 + 2
    cbfd = nc.dram_tensor("cbf", [128, NBF], BF16, kind="ExternalInput").ap()
    cf32d = nc.dram_tensor("cf32", [128, npair + 8], F32, kind="ExternalInput").ap()
    n_wqh_lg = max(0, (NGROUPS - WQH_START) // 2)
    wqh = nc.dram_tensor("wqh", [128, max(1, n_wqh_lg) * 16 * 128], BF16,
                         kind="ExternalInput").ap()
    outd = nc.dram_tensor("out", [D, nb], F32, kind="ExternalOutput").ap()


    with tile.TileContext(nc) as tc:
        with (
            tc.tile_pool(name="const", bufs=1) as cp,
            tc.tile_pool(name="load", bufs=LP_BUFS) as lp,
            tc.tile_pool(name="kt", bufs=KT_BUFS) as ktp,
            tc.tile_pool(name="work", bufs=WP_BUFS) as wp,
            tc.tile_pool(name="h2p", bufs=H2_BUFS) as h2p,
            tc.tile_pool(name="wqhp", bufs=5) as wqp,
            tc.tile_pool(name="wqall", bufs=NGROUPS) as wqap,
            tc.tile_pool(name="outp", bufs=2) as op_,
            tc.tile_pool(name="ps1", bufs=PS1_BUFS, space="PSUM") as ps1,
            tc.tile_pool(name="ps2", bufs=PS2_BUFS, space="PSUM") as ps2,
            tc.tile_pool(name="ps3", bufs=PS3_BUFS, space="PSUM") as ps3,
        ):
            # ---- constants into SBUF (packed: 3 DMAs)
            if PACK_CONSTS:
                c_bf = cp.tile([128, NBF], BF16)
                nc.sync.dma_start(out=c_bf[:], in_=cbfd)
                c_f32 = cp.tile([128, npair + 8], F32)
                nc.sync.dma_start(out=c_f32[:], in_=cf32d)
                c_qw1 = cp.tile([1, 128 * npair], BF16)
                nc.sync.dma_start(out=c_qw1[:], in_=qw1)
                o_ = 0
                c_mask = c_bf[:, o_ : o_ + 32 * NGROUPS]; o_ += 32 * NGROUPS
                c_wk = c_bf[:, o_ : o_ + 128]; o_ += 128
                c_wqk = c_bf[:, o_ : o_ + 128]; o_ += 128
                c_w2b = c_bf[:, o_ : o_ + 32]; o_ += 32
                c_w2a = c_bf[:, o_ : o_ + 32]; o_ += 32
                c_w2na = c_bf[:, o_ : o_ + 32]; o_ += 32
                c_w34 = c_bf[:, o_ : o_ + 2]
                c_qp = c_f32[:, 0:npair]
                c_cols = c_f32[:, npair : npair + 8]
            else:
                c_qp = cp.tile([128, npair], F32)
                nc.sync.dma_start(out=c_qp[:], in_=qp)
                c_qw1 = cp.tile([1, 128 * npair], BF16)
                nc.sync.dma_start(out=c_qw1[:], in_=qw1)
                c_mask = cp.tile([128, 32 * NGROUPS], BF16)
                nc.sync.dma_start(out=c_mask[:], in_=maskd)
                c_wk = cp.tile([128, 128], BF16)
                nc.sync.dma_start(out=c_wk[:], in_=wk2)
                c_wqk = cp.tile([128, 128], BF16)
                nc.sync.dma_start(out=c_wqk[:], in_=wqk2)
                c_w2b = cp.tile([128, 32], BF16)
                nc.sync.dma_start(out=c_w2b[:], in_=w2b)
                c_w2a = cp.tile([128, 32], BF16)
                nc.sync.dma_start(out=c_w2a[:], in_=w2a)
                c_w2na = cp.tile([128, 32], BF16)
                nc.sync.dma_start(out=c_w2na[:], in_=w2na)
                c_w34 = cp.tile([128, 2], BF16)
                nc.sync.dma_start(out=c_w34[:], in_=w34)
                c_cols = cp.tile([128, 8], F32)
                nc.sync.dma_start(out=c_cols[:], in_=colsb)
            c_ones = cp.tile([1, 128], BF16)
            nc.gpsimd.memset(c_ones[:], 1.0)


            wq_all = {}
            if HOIST_WQ:
                for g2 in range(min(NGROUPS, HOIST_N)):
                    wqv = wqap.tile([128, 8, 128], BF16, tag="wq")
                    for j in range(8):
                        pj = 8 * g2 + j
                        nc.vector.tensor_scalar(
                            wqv[:, j, :], c_wqk[:], c_qp[:, pj : pj + 1], None, ALU.mult
                        )
                    wq_all[g2] = wqv

            unit_ctr = 0
            gf_by_lg = {}
            kt_by_lg = {}
            wqh_by_lg = {}
            po_sb = None
            order = COMPUTE_ORDER if COMPUTE_ORDER else tuple(range(NGROUPS))
            done_in_blk = {}
            po_tiles = {}
            for gi, g in enumerate(order):
                T1, T2 = sched[g]
                Teff = T1 + T2
                lb = g // 4              # 64-row load block
                lg = g // 2              # 32-row transpose pair
                rbase = 64 * lb
                ro = 16 * (g % 4)        # row offset within load block

                if lb not in gf_by_lg:
                    TL1 = max(sched[i][0] for i in range(4 * lb, 4 * lb + 4))
                    TL2 = max(sched[i][1] for i in range(4 * lb, 4 * lb + 4))
                    boff = lb * ST * 64 * D
                    gfa = lp.tile([128, 2, 64, 64], BF16, tag="gf")
                    nc.gpsimd.dma_start(
                        out=gfa[0:TL1, 0, :, :],
                        in_=bass.AP(keyt.tensor, boff,
                                    [[64 * D, TL1], [D, 64], [1, D]]),
                    )
                    if TL2 > 0:
                        nc.gpsimd.dma_start(
                            out=gfa[0:TL2, 1, :, :],
                            in_=bass.AP(keyt.tensor, boff + 128 * 64 * D,
                                        [[64 * D, TL2], [D, 64], [1, D]]),
                        )
                    gf_by_lg[lb] = gfa
                gfa = gf_by_lg[lb]

                # ---- transposes at 32-row (2-group) granularity
                if lg not in kt_by_lg:
                    T1lg = max(sched[2 * lg][0], sched[2 * lg + 1][0])
                    T2lg = max(sched[2 * lg][1], sched[2 * lg + 1][1])
                    tro = 32 * (lg % 2)
                    eng1 = nc.sync if lg % 2 == 0 else nc.scalar
                    eng2 = nc.scalar if lg % 2 == 0 else nc.sync
                    ktf = ktp.tile([128, 16, 128], BF16, tag="ktf")
                    eng1.dma_start(
                        out=ktf[:, :, 0:T1lg],
                        in_=gfa[0:T1lg, 0, tro : tro + 32, :].rearrange("p b d -> p (b d)"),
                        transpose=True,
                    )
                    ktq = None
                    if T2lg > 0:
                        ktq = ktp.tile([128, 16, 80], BF16, tag="ktq")
                        eng2.dma_start(
                            out=ktq[:, :, 0:T2lg],
                            in_=gfa[0:T2lg, 1, tro : tro + 32, :].rearrange("p b d -> p (b d)"),
                            transpose=True,
                        )
                    kt_by_lg[lg] = (ktf, ktq)
                ktf, ktq = kt_by_lg[lg]
                jo = 8 * (g % 2)         # pair index offset within ktf/ktq

                # ---- per-pair q∘Wqk: DVE-built early, host-loaded late
                if HOIST_WQ and g in wq_all:
                    wq = wq_all[g]
                elif g >= WQH_START:
                    if lg not in wqh_by_lg:
                        wqt = wqp.tile([128, 16, 128], BF16, tag="wqh")
                        lgq = lg - WQH_START // 2
                        (nc.sync if lg % 2 else nc.scalar).dma_start(
                            out=wqt[:],
                            in_=wqh[:, 2048 * lgq : 2048 * (lgq + 1)],
                        )
                        wqh_by_lg[lg] = wqt
                    wq = wqh_by_lg[lg][:, 8 * (g % 2) : 8 * (g % 2) + 8, :]
                else:
                    wqv = wp.tile([128, 8, 128], BF16, tag="wq")
                    for j in range(8):
                        pj = 8 * g + j
                        nc.vector.tensor_scalar(
                            wqv[:, j, :], c_wqk[:], c_qp[:, pj : pj + 1], None, ALU.mult
                        )
                    wq = wqv

                # ---- layer 1 + dice1 in units of `up` pairs
                up = 8 if Teff <= UP8 else (4 if Teff <= UP4 else 2)
                z2 = ps2.tile([128, 416], F32, tag="z2")
                for u in range(8 // up):
                    C = up * Teff
                    z1 = ps1.tile([128, 512], F32, tag="z1")
                    for jj in range(up):
                        j = u * up + jj
                        pj = 8 * g + j
                        o = jj * Teff
                        qwj = c_qw1[0:1, 128 * pj : 128 * pj + 128]
                        nc.tensor.matmul(z1[:, o : o + T1], qwj, c_ones[0:1, 0:T1],
                                         start=True, stop=False)
                        nc.tensor.matmul(z1[:, o : o + T1], c_wk[:], ktf[:, jo + j, 0:T1],
                                         start=False, stop=False)
                        nc.tensor.matmul(z1[:, o : o + T1], wq[:, j, :], ktf[:, jo + j, 0:T1],
                                         start=False, stop=True)
                        if T2 > 0:
                            nc.tensor.matmul(z1[:, o + T1 : o + Teff], qwj,
                                             c_ones[0:1, 0:T2], start=True, stop=False)
                            nc.tensor.matmul(z1[:, o + T1 : o + Teff], c_wk[:],
                                             ktq[:, jo + j, 0:T2], start=False, stop=False)
                            nc.tensor.matmul(z1[:, o + T1 : o + Teff], wq[:, j, :],
                                             ktq[:, jo + j, 0:T2], start=False, stop=True)

                    p1t = wp.tile([128, 512], BF16, tag="p1")
                    nc.scalar.activation(p1t[:, 0:C], z1[:, 0:C], AF.Sigmoid,
                                         bias=c_cols[:, 1:2], scale=c_cols[:, 0:1])
                    r2b = (unit_ctr * R2B_NUM) % R2B_DEN < R2B_NUM
                    unit_ctr += 1
                    if r2b:
                        x1c = wp.tile([128, 512], BF16, tag="x1c")
                        nc.scalar.copy(x1c[:, 0:C], z1[:, 0:C])
                        u1t = wp.tile([128, 512], BF16, tag="u1")
                        nc.vector.tensor_tensor(u1t[:, 0:C], x1c[:, 0:C], p1t[:, 0:C],
                                                ALU.mult)
                        for jj in range(up):
                            j = u * up + jj
                            o = jj * Teff
                            b = j // 2
                            co = (j % 2) * Teff
                            nc.tensor.matmul(z2[32 * b : 32 * b + 32, co : co + Teff],
                                             c_w2a[:], x1c[:, o : o + Teff],
                                             start=True, stop=False,
                                             tile_position=(0, 32 * b))
                            nc.tensor.matmul(z2[32 * b : 32 * b + 32, co : co + Teff],
                                             c_w2na[:], u1t[:, o : o + Teff],
                                             start=False, stop=True,
                                             tile_position=(0, 32 * b))
                    else:
                        p1m = wp.tile([128, 512], BF16, tag="p1m")
                        nc.vector.tensor_scalar(p1m[:, 0:C], p1t[:, 0:C],
                                                c_cols[:, 2:3], c_cols[:, 3:4],
                                                ALU.mult, ALU.add)
                        h1t = wp.tile([128, 512], BF16, tag="h1")
                        nc.vector.tensor_tensor(h1t[:, 0:C], z1[:, 0:C], p1m[:, 0:C],
                                                ALU.mult)
                        for jj in range(up):
                            j = u * up + jj
                            o = jj * Teff
                            b = j // 2
                            co = (j % 2) * Teff
                            nc.tensor.matmul(z2[32 * b : 32 * b + 32, co : co + Teff],
                                             c_w2b[:], h1t[:, o : o + Teff],
                                             start=True, stop=True,
                                             tile_position=(0, 32 * b))

                # ---- dice2, batched over the whole group
                C2 = 2 * Teff
                p2t = wp.tile([128, 448], BF16, tag="p2")
                nc.scalar.activation(p2t[:, 0:C2], z2[:, 0:C2], AF.Sigmoid,
                                     bias=c_cols[:, 5:6], scale=c_cols[:, 4:5])
                t2t = wp.tile([128, 448], BF16, tag="t2")
                nc.vector.tensor_scalar(t2t[:, 0:C2], p2t[:, 0:C2], c_cols[:, 6:7],
                                        c_cols[:, 7:8], ALU.mult, ALU.add)
                h2t = h2p.tile([128, 448], BF16, tag="h2")
                if g % H2B_MOD == H2B_PHASE:
                    x2c = wp.tile([128, 448], BF16, tag="x2c")
                    nc.scalar.copy(x2c[:, 0:C2], z2[:, 0:C2])
                    nc.vector.tensor_tensor(h2t[:, 0:C2], x2c[:, 0:C2], t2t[:, 0:C2],
                                            ALU.mult)
                else:
                    nc.vector.tensor_tensor(h2t[:, 0:C2], z2[:, 0:C2], t2t[:, 0:C2],
                                            ALU.mult)

                # ---- scores (token-major PSUM [128, 32])
                sc = ps3.tile([128, 48], F32, tag="sc")
                nc.vector.memset(sc[:, 0:32], 0.0)
                for j in range(8):
                    b = j // 2
                    co = (j % 2) * Teff
                    nc.tensor.matmul(sc[0:T1, 4 * j : 4 * j + 2],
                                     h2t[32 * b : 32 * b + 32, co : co + T1],
                                     c_w34[32 * b : 32 * b + 32, :],
                                     start=True, stop=True,
                                     tile_position=(32 * b, 0))
                    if T2 > 0:
                        nc.tensor.matmul(sc[0:T2, 4 * j + 2 : 4 * j + 4],
                                         h2t[32 * b : 32 * b + 32, co + T1 : co + Teff],
                                         c_w34[32 * b : 32 * b + 32, :],
                                         start=True, stop=True,
                                         tile_position=(32 * b, 0))

                sg = wp.tile([128, 32], BF16, tag="sg")
                nc.scalar.activation(sg[:], sc[:, 0:32], AF.Sigmoid)
                wt = wp.tile([128, 32], BF16, tag="wt")
                nc.vector.tensor_tensor(wt[:], sg[:], c_mask[:, 32 * g : 32 * g + 32],
                                        ALU.mult)

                # ---- pooling: out[d, row] = k^T w, N=1 matmuls
                po = sc[0:64, 32:48]
                for r in range(16):
                    j = r // 2
                    bh = r % 2
                    nc.tensor.matmul(po[:, r : r + 1], gfa[0:T1, 0, ro + r, :],
                                     wt[0:T1, 4 * j + bh : 4 * j + bh + 1],
                                     start=True, stop=(T2 == 0))
                    if T2 > 0:
                        nc.tensor.matmul(po[:, r : r + 1], gfa[0:T2, 1, ro + r, :],
                                         wt[0:T2, 4 * j + 2 + bh : 4 * j + 3 + bh],
                                         start=False, stop=True)

                ob = g // 4
                if ob not in po_tiles:
                    po_sb = op_.tile([64, 64], F32, tag="posb")
                    po_tiles[ob] = po_sb
                    done_in_blk[ob] = 0
                po_sb = po_tiles[ob]
                if POCOPY_ACT:
                    nc.scalar.copy(po_sb[:, 16 * (g % 4) : 16 * (g % 4) + 16], po)
                else:
                    nc.vector.tensor_copy(po_sb[:, 16 * (g % 4) : 16 * (g % 4) + 16], po)
                done_in_blk[ob] += 1
                if done_in_blk[ob] == 4:
                    nc.sync.dma_start(
                        out=outd[:, 64 * ob : 64 * ob + 64], in_=po_sb[:]
                    )
    nc.compile()
    return nc


def _blk(a):
    m = np.zeros((128, 2 * a.shape[1]), np.float32)
    m[0:64, 0 : a.shape[1]] = a
    m[64:128, a.shape[1] :] = a
    return m


def _prep_consts(W1, alpha1, mean1, var1, W2, alpha2, mean2, var2, W3):
    inv1 = 1.0 / np.sqrt(var1 + EPS)
    inv2 = 1.0 / np.sqrt(var2 + EPS)
    Wq = W1[0:64] + W1[128:192]
    Wk = W1[64:128] - W1[128:192]
    Wqk = W1[192:256]

    wk2 = _blk(Wk).astype(bf)
    wqk2 = _blk(Wqk).astype(bf)
    w2b = _blk(W2).astype(bf)
    w2a = _blk(np.diag(alpha1) @ W2).astype(bf)
    w2na = _blk(np.diag(1.0 - alpha1) @ W2).astype(bf)
    w34p = np.zeros((32, 2), np.float32)
    w34p[0:16, 0] = W3[:, 0]
    w34p[16:32, 1] = W3[:, 0]
    w34 = np.tile(w34p, (4, 1)).astype(bf)
    colsb = np.zeros((128, 8), np.float32)
    colsb[:, 0] = np.tile(inv1, 2)
    colsb[:, 1] = np.tile(-mean1 * inv1, 2)
    colsb[:, 2] = np.tile(1.0 - alpha1, 2)
    colsb[:, 3] = np.tile(alpha1, 2)
    colsb[:, 4] = np.tile(inv2, 8)
    colsb[:, 5] = np.tile(-mean2 * inv2, 8)
    colsb[:, 6] = np.tile(1.0 - alpha2, 8)
    colsb[:, 7] = np.tile(alpha2, 8)
    return Wq, wk2, wqk2, w2b, w2a, w2na, w34, colsb


def kernel(query_emb, key_emb, seq_length, W1, alpha1, mean1, var1,
           W2, alpha2, mean2, var2, W3):
    (Wq, wk2, wqk2, w2b, w2a, w2na, w34, colsb) = _prep_consts(
        np.asarray(W1, np.float32), np.asarray(alpha1, np.float32),
        np.asarray(mean1, np.float32), np.asarray(var1, np.float32),
        np.asarray(W2, np.float32), np.asarray(alpha2, np.float32),
        np.asarray(mean2, np.float32), np.asarray(var2, np.float32),
        np.asarray(W3, np.float32))
    q = np.asarray(query_emb, np.float32)
    k = np.asarray(key_emb, np.float32)
    sl = np.asarray(seq_length).reshape(-1).astype(np.int64)

    qW = (q @ Wq).astype(np.float32)  # [B, 64]

    order = np.argsort(sl, kind="stable")
    shards = [order[c::NCORES] for c in range(NCORES)]

    sched = []
    for g in range(NGROUPS):
        mx = max(int(sl[shards[c][16 * g : 16 * g + 16]].max()) for c in range(NCORES))
        sched.append((min(128, _ceil16(mx)), _ceil16(max(0, mx - 128))))
    sched = tuple(sched)

    if sched not in _CACHE:
        _CACHE[sched] = _build(sched)
    nc = _CACHE[sched]
    npair = BLOC // 2

    t_full = np.arange(128)[:, None]
    t_part = np.arange(128)[:, None] + 128

    in_maps = []
    for c in range(NCORES):
        rows = shards[c]
        slc = sl[rows]
        qs = q[rows]          # [256, 64]
        qWs = qW[rows]        # [256, 64]

        qp_t = np.zeros((128, npair), np.float32)
        qp_t[0:64] = qs[0::2].T
        qp_t[64:128] = qs[1::2].T

        qw1_t = np.zeros((1, 128 * npair), np.float32)
        qw1_r = qw1_t.reshape(npair, 2, 64)
        qw1_r[:, 0, :] = qWs[0::2]
        qw1_r[:, 1, :] = qWs[1::2]

        mk = np.zeros((128, 32 * NGROUPS), np.float32)
        for g in range(NGROUPS):
            sg_ = slc[16 * g : 16 * g + 16]
            full = (t_full < sg_[None, :]).astype(np.float32)   # [128, 16]
            part = (t_part < sg_[None, :]).astype(np.float32)
            mk[:, 32 * g + 0 : 32 * g + 32 : 4] = full[:, 0::2]
            mk[:, 32 * g + 1 : 32 * g + 32 : 4] = full[:, 1::2]
            mk[:, 32 * g + 2 : 32 * g + 32 : 4] = part[:, 0::2]
            mk[:, 32 * g + 3 : 32 * g + 32 : 4] = part[:, 1::2]

        ks_ = k[rows]  # [256, 200, 64]
        keyt_h = np.zeros((4, 256, 64, D), np.float32)
        for b_ in range(4):
            keyt_h[b_, 0:S] = ks_[64 * b_ : 64 * b_ + 64].transpose(1, 0, 2)
        n_wqh_lg = max(0, (NGROUPS - WQH_START) // 2)
        if n_wqh_lg > 0:
            wq_pairs = qp_t[:, 8 * WQH_START :]                      # [128, npairs_late]
            wqk_f = np.asarray(wqk2, np.float32)
            wqh_h = (wq_pairs[:, :, None] * wqk_f[:, None, :]).astype(bf)
            wqh_h = wqh_h.reshape(128, n_wqh_lg * 16 * 128)
        else:
            wqh_h = np.zeros((128, 1 * 16 * 128), bf)
        cbf_h = np.concatenate([mk, wk2, wqk2, w2b, w2a, w2na, w34], axis=1).astype(bf)
        cf32_h = np.concatenate([qp_t, colsb], axis=1).astype(np.float32)
        in_maps.append({
            "cbf": cbf_h, "cf32": cf32_h,
            "wqh": wqh_h,
            "keyt": keyt_h.reshape(4 * 256 * 64, D),
            "qp": qp_t,
            "qw1": qw1_t.astype(bf),
            "maskd": mk.astype(bf),
            "wk2": wk2, "wqk2": wqk2, "w2b": w2b, "w2a": w2a, "w2na": w2na,
            "w34": w34, "colsb": colsb,
        })

    res = run_bass_kernel_spmd(nc, in_maps, list(range(NCORES)), trace=TRACE)
    global LAST_RESULT
    LAST_RESULT = res

    out_full = np.zeros((B, D), np.float32)
    for c in range(NCORES):
        out_full[shards[c]] = np.asarray(res.results[c]["out"], np.float32).T
    return out_full


# revision 51
# speedup vs baseline: 1.1070x; 1.1070x over previous
"""AttentionSequencePoolingLayer Trainium2 kernel (8-core data parallel), v2.

B=2048, S=200, D=64, H1=64, H2=16. Batch sharded 256/core.

Strategy:
- Rows are globally sorted by seq_length and dealt round-robin to the 8 cores,
  so every core sees the same length profile. Within a core, rows are grouped
  16 at a time; group g only processes T_g = roundup(max seq_length, 16)
  tokens (ΣT ≈ 0.57 × S). One program (compiled per schedule) serves all cores.
- k is re-laid token-major per 64-row block on the host (8KB DMA descriptors,
  zero-padded to 256 tokens); cast-load bf16 (tokens on partitions), xbar-transpose
  to kT [(bhat,d), tok] pair tiles.
- z1 = x1 = att@W1 lands in PSUM via 3 accumulating matmuls per pair:
  qW ⊗ ones (K=1 rank-1), Wk^T kT, (q∘Wqk)^T kT. The qW fold makes the
  dice-1 sigmoid bias a shared constant, so ACT/DVE ops batch across pairs.
- dice1, two routes (mixed 3:8 r2b:r1 to balance ACT vs DVE):
  r1: ACT p1=σ(s·x1+b); DVE p1~=α+(1-α)p1; DVE h1=x1(PSUM)∘p1~; z2=W2^T h1.
  r2b: ACT p1; ACT copy x1→SBUF; DVE u1=x1∘p1; z2 = (αW2)^T x1 + ((1-α)W2)^T u1.
- dice2 batched over all 8 pairs of a group [128=(pair,bhat,h2), 2T].
- scores via tiny N=2 matmuls into token-major PSUM; σ+mask; pooling as
  out[d,1] = k_nat^T w with N=1 matmuls (k stationary); d-major DRAM output,
  transposed on host at gather.
"""
import numpy as np
import ml_dtypes

import concourse.bacc as bacc
import concourse.tile as tile
import concourse.mybir as mybir
import concourse.bass as bass
from concourse.bass_utils import run_bass_kernel_spmd

B, S, D = 2048, 200, 64
H1, H2 = 64, 16
EPS = 1e-9
NCORES = 8
BLOC = B // NCORES          # 256 rows per core
NGROUPS = BLOC // 16        # 16

F32 = mybir.dt.float32
BF16 = mybir.dt.bfloat16
AF = mybir.ActivationFunctionType
ALU = mybir.AluOpType
bf = ml_dtypes.bfloat16

# fraction of z1-units on route-2b (ACT-heavy) vs route-1 (DVE-heavy): num/den
R2B_NUM, R2B_DEN = 3, 8
LP_BUFS, KT_BUFS, WP_BUFS, H2_BUFS = 2, 6, 6, 3
PS1_BUFS, PS2_BUFS, PS3_BUFS = 3, 2, 3
WQ_ACT_MOD = 9  # j%4 value routed to ACT; 9=never
UP8, UP4 = 56, 112
WQ_POOL_MOD = 9
PACK_CONSTS = False
HOIST_WQ = False
HOIST_N = 16
TSWAP = False
CSPLIT = False
POCOPY_ACT = False
H2B_MOD, H2B_PHASE = 99, 98
WQH_START = 99  # groups >= this use host-precomputed q*Wqk (disabled: adds HWDGE sem pressure)
COMPUTE_ORDER = ()
LOAD_BLOCKS = ((0, 4), (4, 4), (8, 4), (12, 4))  # (start_group, ngroups)

_CACHE = {}
TRACE = False
LAST_RESULT = None


def _ceil16(x):
    return (int(x) + 15) // 16 * 16


def _build(sched):
    nc = bacc.Bacc("TRN2", target_bir_lowering=False, debug=False, num_devices=NCORES,
                   dynamic_dma_scratch_size=16384)
    nb = BLOC
    npair = nb // 2  # 128

    ST = 256  # padded tokens per load block (two 128-token chunks)
    keyt = nc.dram_tensor("keyt", [4 * ST * 64, D], F32, kind="ExternalInput").ap()
    qp = nc.dram_tensor("qp", [128, npair], F32, kind="ExternalInput").ap()
    qw1 = nc.dram_tensor("qw1", [1, 128 * npair], BF16, kind="ExternalInput").ap()
    maskd = nc.dram_tensor("maskd", [128, 32 * NGROUPS], BF16, kind="ExternalInput").ap()
    wk2 = nc.dram_tensor("wk2", [128, 128], BF16, kind="ExternalInput").ap()
    wqk2 = nc.dram_tensor("wqk2", [128, 128], BF16, kind="ExternalInput").ap()
    w2b = nc.dram_tensor("w2b", [128, 32], BF16, kind="ExternalInput").ap()
    w2a = nc.dram_tensor("w2a", [128, 32], BF16, kind="ExternalInput").ap()
    w2na = nc.dram_tensor("w2na", [128, 32], BF16, kind="ExternalInput").ap()
    w34 = nc.dram_tensor("w34", [128, 2], BF16, kind="ExternalInput").ap()
    colsb = nc.dram_tensor("colsb", [128, 8], F32, kind="ExternalInput").ap()
    NBF = 32 * NGROUPS + 128 + 128 + 32 + 32 + 32 + 2
    cbfd = nc.dram_tensor("cbf", [128, NBF], BF16, kind="ExternalInput").ap()
    cf32d = nc.dram_tensor("cf32", [128, npair + 8], F32, kind="ExternalInput").ap()
    n_wqh_lg = max(0, (NGROUPS - WQH_START) // 2)
    wqh = nc.dram_tensor("wqh", [128, max(1, n_wqh_lg) * 16 * 128], BF16,
                         kind="ExternalInput").ap()
    outd = nc.dram_tensor("out", [D, nb], F32, kind="ExternalOutput").ap()


    with tile.TileContext(nc) as tc:
        with (
            tc.tile_pool(name="const", bufs=1) as cp,
            tc.tile_pool(name="load", bufs=LP_BUFS) as lp,
            tc.tile_pool(name="kt", bufs=KT_BUFS) as ktp,
            tc.tile_pool(name="work", bufs=WP_BUFS) as wp,
            tc.tile_pool(name="h2p", bufs=H2_BUFS) as h2p,
            tc.tile_pool(name="wqhp", bufs=5) as wqp,
            tc.tile_pool(name="wqall", bufs=NGROUPS) as wqap,
            tc.tile_pool(name="outp", bufs=2) as op_,
            tc.tile_pool(name="ps1", bufs=PS1_BUFS, space="PSUM") as ps1,
            tc.tile_pool(name="ps2", bufs=PS2_BUFS, space="PSUM") as ps2,
            tc.tile_pool(name="ps3", bufs=PS3_BUFS, space="PSUM") as ps3,
        ):
            # ---- constants into SBUF (packed: 3 DMAs)
            if PACK_CONSTS:
                c_bf = cp.tile([128, NBF], BF16)
                nc.sync.dma_start(out=c_bf[:], in_=cbfd)
                c_f32 = cp.tile([128, npair + 8], F32)
                nc.sync.dma_start(out=c_f32[:], in_=cf32d)
                c_qw1 = cp.tile([1, 128 * npair], BF16)
                nc.sync.dma_start(out=c_qw1[:], in_=qw1)
                o_ = 0
                c_mask = c_bf[:, o_ : o_ + 32 * NGROUPS]; o_ += 32 * NGROUPS
                c_wk = c_bf[:, o_ : o_ + 128]; o_ += 128
                c_wqk = c_bf[:, o_ : o_ + 128]; o_ += 128
                c_w2b = c_bf[:, o_ : o_ + 32]; o_ += 32
                c_w2a = c_bf[:, o_ : o_ + 32]; o_ += 32
                c_w2na = c_bf[:, o_ : o_ + 32]; o_ += 32
                c_w34 = c_bf[:, o_ : o_ + 2]
                c_qp = c_f32[:, 0:npair]
                c_cols = c_f32[:, npair : npair + 8]
            else:
                ce = [nc.sync, nc.scalar] if CSPLIT else [nc.sync, nc.sync]
                c_qp = cp.tile([128, npair], F32)
                ce[0].dma_start(out=c_qp[:], in_=qp)
                c_qw1 = cp.tile([1, 128 * npair], BF16)
                ce[1].dma_start(out=c_qw1[:], in_=qw1)
                c_mask = cp.tile([128, 32 * NGROUPS], BF16)
                ce[0].dma_start(out=c_mask[:], in_=maskd)
                c_wk = cp.tile([128, 128], BF16)
                ce[1].dma_start(out=c_wk[:], in_=wk2)
                c_wqk = cp.tile([128, 128], BF16)
                ce[0].dma_start(out=c_wqk[:], in_=wqk2)
                c_w2b = cp.tile([128, 32], BF16)
                ce[1].dma_start(out=c_w2b[:], in_=w2b)
                c_w2a = cp.tile([128, 32], BF16)
                ce[0].dma_start(out=c_w2a[:], in_=w2a)
                c_w2na = cp.tile([128, 32], BF16)
                ce[1].dma_start(out=c_w2na[:], in_=w2na)
                c_w34 = cp.tile([128, 2], BF16)
                ce[0].dma_start(out=c_w34[:], in_=w34)
                c_cols = cp.tile([128, 8], F32)
                ce[1].dma_start(out=c_cols[:], in_=colsb)
            c_ones = cp.tile([1, 128], BF16)
            nc.gpsimd.memset(c_ones[:], 1.0)


            wq_all = {}
            if HOIST_WQ:
                for g2 in range(min(NGROUPS, HOIST_N)):
                    wqv = wqap.tile([128, 8, 128], BF16, tag="wq")
                    for j in range(8):
                        pj = 8 * g2 + j
                        nc.vector.tensor_scalar(
                            wqv[:, j, :], c_wqk[:], c_qp[:, pj : pj + 1], None, ALU.mult
                        )
                    wq_all[g2] = wqv

            unit_ctr = 0
            gf_by_lg = {}
            kt_by_lg = {}
            wqh_by_lg = {}
            po_sb = None
            order = COMPUTE_ORDER if COMPUTE_ORDER else tuple(range(NGROUPS))
            done_in_blk = {}
            po_tiles = {}
            for gi, g in enumerate(order):
                T1, T2 = sched[g]
                Teff = T1 + T2
                lb = g // 4              # 64-row load block
                lg = g // 2              # 32-row transpose pair
                rbase = 64 * lb
                ro = 16 * (g % 4)        # row offset within load block

                if lb not in gf_by_lg:
                    TL1 = max(sched[i][0] for i in range(4 * lb, 4 * lb + 4))
                    TL2 = max(sched[i][1] for i in range(4 * lb, 4 * lb + 4))
                    boff = lb * ST * 64 * D
                    gfa = lp.tile([128, 2, 64, 64], BF16, tag="gf")
                    nc.gpsimd.dma_start(
                        out=gfa[0:TL1, 0, :, :],
                        in_=bass.AP(keyt.tensor, boff,
                                    [[64 * D, TL1], [D, 64], [1, D]]),
                    )
                    if TL2 > 0:
                        nc.gpsimd.dma_start(
                            out=gfa[0:TL2, 1, :, :],
                            in_=bass.AP(keyt.tensor, boff + 128 * 64 * D,
                                        [[64 * D, TL2], [D, 64], [1, D]]),
                        )
                    gf_by_lg[lb] = gfa
                gfa = gf_by_lg[lb]

                # ---- transposes at 32-row (2-group) granularity
                if lg not in kt_by_lg:
                    T1lg = max(sched[2 * lg][0], sched[2 * lg + 1][0])
                    T2lg = max(sched[2 * lg][1], sched[2 * lg + 1][1])
                    tro = 32 * (lg % 2)
                    eng1 = (nc.scalar if lg % 2 == 0 else nc.sync) if TSWAP else (nc.sync if lg % 2 == 0 else nc.scalar)
                    eng2 = (nc.sync if lg % 2 == 0 else nc.scalar) if TSWAP else (nc.scalar if lg % 2 == 0 else nc.sync)
                    ktf = ktp.tile([128, 16, 128], BF16, tag="ktf")
                    eng1.dma_start(
                        out=ktf[:, :, 0:T1lg],
                        in_=gfa[0:T1lg, 0, tro : tro + 32, :].rearrange("p b d -> p (b d)"),
                        transpose=True,
                    )
                    ktq = None
                    if T2lg > 0:
                        ktq = ktp.tile([128, 16, 80], BF16, tag="ktq")
                        eng2.dma_start(
                            out=ktq[:, :, 0:T2lg],
                            in_=gfa[0:T2lg, 1, tro : tro + 32, :].rearrange("p b d -> p (b d)"),
                            transpose=True,
                        )
                    kt_by_lg[lg] = (ktf, ktq)
                ktf, ktq = kt_by_lg[lg]
                jo = 8 * (g % 2)         # pair index offset within ktf/ktq

                # ---- per-pair q∘Wqk: DVE-built early, host-loaded late
                if HOIST_WQ and g in wq_all:
                    wq = wq_all[g]
                elif g >= WQH_START:
                    if lg not in wqh_by_lg:
                        wqt = wqp.tile([128, 16, 128], BF16, tag="wqh")
                        lgq = lg - WQH_START // 2
                        (nc.sync if lg % 2 else nc.scalar).dma_start(
                            out=wqt[:],
                            in_=wqh[:, 2048 * lgq : 2048 * (lgq + 1)],
                        )
                        wqh_by_lg[lg] = wqt
                    wq = wqh_by_lg[lg][:, 8 * (g % 2) : 8 * (g % 2) + 8, :]
                else:
                    wqv = wp.tile([128, 8, 128], BF16, tag="wq")
                    for j in range(8):
                        pj = 8 * g + j
                        nc.vector.tensor_scalar(
                            wqv[:, j, :], c_wqk[:], c_qp[:, pj : pj + 1], None, ALU.mult
                        )
                    wq = wqv

                # ---- layer 1 + dice1 in units of `up` pairs
                up = 8 if Teff <= UP8 else (4 if Teff <= UP4 else 2)
                z2 = ps2.tile([128, 416], F32, tag="z2")
                for u in range(8 // up):
                    C = up * Teff
                    z1 = ps1.tile([128, 512], F32, tag="z1")
                    for jj in range(up):
                        j = u * up + jj
                        pj = 8 * g + j
                        o = jj * Teff
                        qwj = c_qw1[0:1, 128 * pj : 128 * pj + 128]
                        nc.tensor.matmul(z1[:, o : o + T1], qwj, c_ones[0:1, 0:T1],
                                         start=True, stop=False)
                        nc.tensor.matmul(z1[:, o : o + T1], c_wk[:], ktf[:, jo + j, 0:T1],
                                         start=False, stop=False)
                        nc.tensor.matmul(z1[:, o : o + T1], wq[:, j, :], ktf[:, jo + j, 0:T1],
                                         start=False, stop=True)
                        if T2 > 0:
                            nc.tensor.matmul(z1[:, o + T1 : o + Teff], qwj,
                                             c_ones[0:1, 0:T2], start=True, stop=False)
                            nc.tensor.matmul(z1[:, o + T1 : o + Teff], c_wk[:],
                                             ktq[:, jo + j, 0:T2], start=False, stop=False)
                            nc.tensor.matmul(z1[:, o + T1 : o + Teff], wq[:, j, :],
                                             ktq[:, jo + j, 0:T2], start=False, stop=True)

                    p1t = wp.tile([128, 512], BF16, tag="p1")
                    nc.scalar.activation(p1t[:, 0:C], z1[:, 0:C], AF.Sigmoid,
                                         bias=c_cols[:, 1:2], scale=c_cols[:, 0:1])
                    r2b = (unit_ctr * R2B_NUM) % R2B_DEN < R2B_NUM
                    unit_ctr += 1
                    if r2b:
                        x1c = wp.tile([128, 512], BF16, tag="x1c")
                        nc.scalar.copy(x1c[:, 0:C], z1[:, 0:C])
                        u1t = wp.tile([128, 512], BF16, tag="u1")
                        nc.vector.tensor_tensor(u1t[:, 0:C], x1c[:, 0:C], p1t[:, 0:C],
                                                ALU.mult)
                        for jj in range(up):
                            j = u * up + jj
                            o = jj * Teff
                            b = j // 2
                            co = (j % 2) * Teff
                            nc.tensor.matmul(z2[32 * b : 32 * b + 32, co : co + Teff],
                                             c_w2a[:], x1c[:, o : o + Teff],
                                             start=True, stop=False,
                                             tile_position=(0, 32 * b))
                            nc.tensor.matmul(z2[32 * b : 32 * b + 32, co : co + Teff],
                                             c_w2na[:], u1t[:, o : o + Teff],
                                             start=False, stop=True,
                                             tile_position=(0, 32 * b))
                    else:
                        p1m = wp.tile([128, 512], BF16, tag="p1m")
                        nc.vector.tensor_scalar(p1m[:, 0:C], p1t[:, 0:C],
                                                c_cols[:, 2:3], c_cols[:, 3:4],
                                                ALU.mult, ALU.add)
                        h1t = wp.tile([128, 512], BF16, tag="h1")
                        nc.vector.tensor_tensor(h1t[:, 0:C], z1[:, 0:C], p1m[:, 0:C],
                                                ALU.mult)
                        for jj in range(up):
                            j = u * up + jj
                            o = jj * Teff
                            b = j // 2
                            co = (j % 2) * Teff
                            nc.tensor.matmul(z2[32 * b : 32 * b + 32, co : co + Teff],
                                             c_w2b[:], h1t[:, o : o + Teff],
                                             start=True, stop=True,
                                             tile_position=(0, 32 * b))

                # ---- dice2, batched over the whole group
                C2 = 2 * Teff
                p2t = wp.tile([128, 448], BF16, tag="p2")
                nc.scalar.activation(p2t[:, 0:C2], z2[:, 0:C2], AF.Sigmoid,
                                     bias=c_cols[:, 5:6], scale=c_cols[:, 4:5])
                t2t = wp.tile([128, 448], BF16, tag="t2")
                nc.vector.tensor_scalar(t2t[:, 0:C2], p2t[:, 0:C2], c_cols[:, 6:7],
                                        c_cols[:, 7:8], ALU.mult, ALU.add)
                h2t = h2p.tile([128, 448], BF16, tag="h2")
                if g % H2B_MOD == H2B_PHASE:
                    x2c = wp.tile([128, 448], BF16, tag="x2c")
                    nc.scalar.copy(x2c[:, 0:C2], z2[:, 0:C2])
                    nc.vector.tensor_tensor(h2t[:, 0:C2], x2c[:, 0:C2], t2t[:, 0:C2],
                                            ALU.mult)
                else:
                    nc.vector.tensor_tensor(h2t[:, 0:C2], z2[:, 0:C2], t2t[:, 0:C2],
                                            ALU.mult)

                # ---- scores (token-major PSUM [128, 32])
                sc = ps3.tile([128, 48], F32, tag="sc")
                nc.vector.memset(sc[:, 0:32], 0.0)
                for j in range(8):
                    b = j // 2
                    co = (j % 2) * Teff
                    nc.tensor.matmul(sc[0:T1, 4 * j : 4 * j + 2],
                                     h2t[32 * b : 32 * b + 32, co : co + T1],
                                     c_w34[32 * b : 32 * b + 32, :],
                                     start=True, stop=True,
                                     tile_position=(32 * b, 0))
                    if T2 > 0:
                        nc.tensor.matmul(sc[0:T2, 4 * j + 2 : 4 * j + 4],
                                         h2t[32 * b : 32 * b + 32, co + T1 : co + Teff],
                                         c_w34[32 * b : 32 * b + 32, :],
                                         start=True, stop=True,
                                         tile_position=(32 * b, 0))

                sg = wp.tile([128, 32], BF16, tag="sg")
                nc.scalar.activation(sg[:], sc[:, 0:32], AF.Sigmoid)
                wt = wp.tile([128, 32], BF16, tag="wt")
                nc.vector.tensor_tensor(wt[:], sg[:], c_mask[:, 32 * g : 32 * g + 32],
                                        ALU.mult)

                # ---- pooling: out[d, row] = k^T w, N=1 matmuls
                po = sc[0:64, 32:48]
                for r in range(16):
                    j = r // 2
                    bh = r % 2
                    nc.tensor.matmul(po[:, r : r + 1], gfa[0:T1, 0, ro + r, :],
                                     wt[0:T1, 4 * j + bh : 4 * j + bh + 1],
                                     start=True, stop=(T2 == 0))
                    if T2 > 0:
                        nc.tensor.matmul(po[:, r : r + 1], gfa[0:T2, 1, ro + r, :],
                                         wt[0:T2, 4 * j + 2 + bh : 4 * j + 3 + bh],
                                         start=False, stop=True)

                ob = g // 4
                if ob not in po_tiles:
                    po_sb = op_.tile([64, 64], F32, tag="posb")
                    po_tiles[ob] = po_sb
                    done_in_blk[ob] = 0
                po_sb = po_tiles[ob]
                if POCOPY_ACT:
                    nc.scalar.copy(po_sb[:, 16 * (g % 4) : 16 * (g % 4) + 16], po)
                else:
                    nc.vector.tensor_copy(po_sb[:, 16 * (g % 4) : 16 * (g % 4) + 16], po)
                done_in_blk[ob] += 1
                if done_in_blk[ob] == 4:
                    nc.sync.dma_start(
                        out=outd[:, 64 * ob : 64 * ob + 64], in_=po_sb[:]
                    )
    nc.compile()
    return nc


def _blk(a):
    m = np.zeros((128, 2 * a.shape[1]), np.float32)
    m[0:64, 0 : a.shape[1]] = a
    m[64:128, a.shape[1] :] = a
    return m


def _prep_consts(W1, alpha1, mean1, var1, W2, alpha2, mean2, var2, W3):
    inv1 = 1.0 / np.sqrt(var1 + EPS)
    inv2 = 1.0 / np.sqrt(var2 + EPS)
    Wq = W1[0:64] + W1[128:192]
    Wk = W1[64:128] - W1[128:192]
    Wqk = W1[192:256]

    wk2 = _blk(Wk).astype(bf)
    wqk2 = _blk(Wqk).astype(bf)
    w2b = _blk(W2).astype(bf)
    w2a = _blk(np.diag(alpha1) @ W2).astype(bf)
    w2na = _blk(np.diag(1.0 - alpha1) @ W2).astype(bf)
    w34p = np.zeros((32, 2), np.float32)
    w34p[0:16, 0] = W3[:, 0]
    w34p[16:32, 1] = W3[:, 0]
    w34 = np.tile(w34p, (4, 1)).astype(bf)
    colsb = np.zeros((128, 8), np.float32)
    colsb[:, 0] = np.tile(inv1, 2)
    colsb[:, 1] = np.tile(-mean1 * inv1, 2)
    colsb[:, 2] = np.tile(1.0 - alpha1, 2)
    colsb[:, 3] = np.tile(alpha1, 2)
    colsb[:, 4] = np.tile(inv2, 8)
    colsb[:, 5] = np.tile(-mean2 * inv2, 8)
    colsb[:, 6] = np.tile(1.0 - alpha2, 8)
    colsb[:, 7] = np.tile(alpha2, 8)
    return Wq, wk2, wqk2, w2b, w2a, w2na, w34, colsb


def kernel(query_emb, key_emb, seq_length, W1, alpha1, mean1, var1,
           W2, alpha2, mean2, var2, W3):
    (Wq, wk2, wqk2, w2b, w2a, w2na, w34, colsb) = _prep_consts(
        np.asarray(W1, np.float32), np.asarray(alpha1, np.float32),
        np.asarray(mean1, np.float32), np.asarray(var1, np.float32),
        np.asarray(W2, np.float32), np.asarray(alpha2, np.float32),
        np.asarray(mean2, np.float32), np.asarray(var2, np.float32),
        np.asarray(W3, np.float32))
    q = np.asarray(query_emb, np.float32)
    k = np.asarray(key_emb, np.float32)
    sl = np.asarray(seq_length).reshape(-1).astype(np.int64)

    qW = (q @ Wq).astype(np.float32)  # [B, 64]

    order = np.argsort(sl, kind="stable")
    shards = [order[c::NCORES] for c in range(NCORES)]

    sched = []
    for g in range(NGROUPS):
        mx = max(int(sl[shards[c][16 * g : 16 * g + 16]].max()) for c in range(NCORES))
        sched.append((min(128, _ceil16(mx)), _ceil16(max(0, mx - 128))))
    sched = tuple(sched)

    if sched not in _CACHE:
        _CACHE[sched] = _build(sched)
    nc = _CACHE[sched]
    npair = BLOC // 2

    t_full = np.arange(128)[:, None]
    t_part = np.arange(128)[:, None] + 128

    in_maps = []
    for c in range(NCORES):
        rows = shards[c]
        slc = sl[rows]
        qs = q[rows]          # [256, 64]
        qWs = qW[rows]        # [256, 64]

        qp_t = np.zeros((128, npair), np.float32)
        qp_t[0:64] = qs[0::2].T
        qp_t[64:128] = qs[1::2].T

        qw1_t = np.zeros((1, 128 * npair), np.float32)
        qw1_r = qw1_t.reshape(npair, 2, 64)
        qw1_r[:, 0, :] = qWs[0::2]
        qw1_r[:, 1, :] = qWs[1::2]

        mk = np.zeros((128, 32 * NGROUPS), np.float32)
        for g in range(NGROUPS):
            sg_ = slc[16 * g : 16 * g + 16]
            full = (t_full < sg_[None, :]).astype(np.float32)   # [128, 16]
            part = (t_part < sg_[None, :]).astype(np.float32)
            mk[:, 32 * g + 0 : 32 * g + 32 : 4] = full[:, 0::2]
            mk[:, 32 * g + 1 : 32 * g + 32 : 4] = full[:, 1::2]
            mk[:, 32 * g + 2 : 32 * g + 32 : 4] = part[:, 0::2]
            mk[:, 32 * g + 3 : 32 * g + 32 : 4] = part[:, 1::2]

        ks_ = k[rows]  # [256, 200, 64]
        keyt_h = np.zeros((4, 256, 64, D), np.float32)
        for b_ in range(4):
            keyt_h[b_, 0:S] = ks_[64 * b_ : 64 * b_ + 64].transpose(1, 0, 2)
        n_wqh_lg = max(0, (NGROUPS - WQH_START) // 2)
        if n_wqh_lg > 0:
            wq_pairs = qp_t[:, 8 * WQH_START :]                      # [128, npairs_late]
            wqk_f = np.asarray(wqk2, np.float32)
            wqh_h = (wq_pairs[:, :, None] * wqk_f[:, None, :]).astype(bf)
            wqh_h = wqh_h.reshape(128, n_wqh_lg * 16 * 128)
        else:
            wqh_h = np.zeros((128, 1 * 16 * 128), bf)
        cbf_h = np.concatenate([mk, wk2, wqk2, w2b, w2a, w2na, w34], axis=1).astype(bf)
        cf32_h = np.concatenate([qp_t, colsb], axis=1).astype(np.float32)
        in_maps.append({
            "cbf": cbf_h, "cf32": cf32_h,
            "wqh": wqh_h,
            "keyt": keyt_h.reshape(4 * 256 * 64, D),
            "qp": qp_t,
            "qw1": qw1_t.astype(bf),
            "maskd": mk.astype(bf),
            "wk2": wk2, "wqk2": wqk2, "w2b": w2b, "w2a": w2a, "w2na": w2na,
            "w34": w34, "colsb": colsb,
        })

    res = run_bass_kernel_spmd(nc, in_maps, list(range(NCORES)), trace=TRACE)
    global LAST_RESULT
    LAST_RESULT = res

    out_full = np.zeros((B, D), np.float32)
    for c in range(NCORES):
        out_full[shards[c]] = np.asarray(res.results[c]["out"], np.float32).T
    return out_full
